# revision 1
# baseline (speedup 1.0000x reference)
"""GCN message-passing kernel for TRN2, 8-core SPMD.

Pipeline per core (destination-sharded):
  x-tilde table build -> AllGather -> L1 aggregate (gather + one-hot matmul)
  -> dense W1 + BN1 + sigmoid -> dense W2 -> h-tilde table -> AllGather
  -> L2 aggregate -> BN2 + sigmoid -> x2^T x2 partial.
Host does integer-only prep: degrees, edge partitioning by destination,
window/chunk schedule, gather index lists, one-hot S blocks, weight/BN
constant folding and bf16 casts.
"""
import math
import numpy as np
import ml_dtypes

import concourse.bacc as bacc
import concourse.bass as bass
import concourse.mybir as mybir
import concourse.tile as tile
from concourse import library_config
from concourse.bass_utils import run_bass_kernel_spmd

BF16 = ml_dtypes.bfloat16
F_IN, F_HID, F_OUT = 128, 256, 128
BN_EPS = 1e-3
GROUP = 8           # chunks per gather group (dma_gather breaks above 1024 idxs)
WD = 64             # dst nodes per aggregation window


class Cfg:
    def __init__(self, n_nodes, n_cores):
        assert n_nodes % n_cores == 0
        self.N = n_nodes
        self.NC = n_cores
        self.NPC = n_nodes // n_cores
        self.HALF = (n_nodes + 1) // 2
        assert self.HALF <= 32768
        self.NDCH = math.ceil(self.NPC / 128)      # 128-row dst chunks
        self.PADD = self.NDCH * 128                # padded local dst count
        self.NW = self.PADD // WD                  # aggregation windows
        assert self.PADD % WD == 0


def _wrap_idx(idx_list):
    """[n] int16 -> [128, n//16] wrapped+replicated layout for dma_gather."""
    n = len(idx_list)
    assert n % 16 == 0
    w = idx_list.reshape(-1, 16).T.astype(np.int16)   # [16, n/16]
    return np.ascontiguousarray(np.tile(w, (8, 1)))   # [128, n/16]


def prep_host(x, edge_index, W1, b1, W2, b2, g1, be1, m1, v1, g2, be2, m2, v2,
              cfg: Cfg):
    """Integer/index preprocessing + parameter folding. Returns
    (in_maps, sched) where sched drives program construction."""
    N, NC, NPC = cfg.N, cfg.NC, cfg.NPC
    src = np.asarray(edge_index[0], dtype=np.int64)
    dst = np.asarray(edge_index[1], dtype=np.int64)

    deg = np.bincount(dst, minlength=N).astype(np.float64) + 1.0
    dinv = (1.0 / np.sqrt(deg)).astype(np.float32)

    # append self loops (src = dst = i)
    allsrc = np.concatenate([src, np.arange(N, dtype=np.int64)])
    alldst = np.concatenate([dst, np.arange(N, dtype=np.int64)])

    core = alldst // NPC
    dloc = alldst % NPC
    win = dloc // WD
    half = (allsrc >= cfg.HALF).astype(np.int64)

    # sort edges by (core, win, half, src) for locality
    order = np.lexsort((allsrc, half, win, core))
    allsrc, core, dloc, win, half = (a[order] for a in (allsrc, core, dloc, win, half))

    # per (core, window, half) edge counts -> common chunk schedule
    NW = cfg.NW
    cnt = np.zeros((NC, NW, 2), dtype=np.int64)
    np.add.at(cnt, (core, win, half), 1)
    nch = np.ceil(cnt / 128).astype(np.int64).max(axis=0)    # [NW, 2]
    nlo_w, nhi_w = nch[:, 0], nch[:, 1]
    NLO, NHI = int(nlo_w.sum()), int(nhi_w.sum())

    # chunk -> window maps (shared across cores)
    sched = {
        "nlo_w": nlo_w, "nhi_w": nhi_w, "NLO": NLO, "NHI": NHI,
    }

    # per-core gather idx lists + S streams
    in_maps = []
    # group edges per core
    edge_core = core
    # precompute per-core per-window per-half slices via searchsorted on the sorted key
    key = ((core * NW + win) * 2 + half)
    # boundaries for every (core, win, half)
    all_keys = np.arange(NC * NW * 2)
    starts = np.searchsorted(key, all_keys, side="left")
    ends = np.searchsorted(key, all_keys, side="right")

    # folded BN constants
    A1 = (g1 * (1.0 / np.sqrt(v1 + BN_EPS))).astype(np.float32)
    B1 = (be1 - m1 * A1).astype(np.float32)
    A2 = (g2 * (1.0 / np.sqrt(v2 + BN_EPS))).astype(np.float32)
    B2 = (be2 - m2 * A2).astype(np.float32)

    # bnc layout [128, 9]: A1a A1b B1a B1b b1a b1b b2 A2 B2
    bnc = np.zeros((128, 9), dtype=np.float32)
    bnc[:, 0], bnc[:, 1] = A1[:128], A1[128:]
    bnc[:, 2], bnc[:, 3] = B1[:128], B1[128:]
    bnc[:, 4], bnc[:, 5] = b1[:128], b1[128:]
    bnc[:, 6], bnc[:, 7], bnc[:, 8] = b2, A2, B2

    W1b = np.asarray(W1, dtype=np.float32).astype(BF16)             # [128, 256]
    # W2sb [128, 2*128]: [p, h*128+f] = W2[h*128+p, f]
    W2f = np.asarray(W2, dtype=np.float32)
    W2sb = np.zeros((128, 256), dtype=np.float32)
    W2sb[:, 0:128] = W2f[0:128, :]
    W2sb[:, 128:256] = W2f[128:256, :]
    W2sb = W2sb.astype(BF16)
    ident = np.eye(128, dtype=np.float32).astype(BF16)

    xf = np.asarray(x, dtype=np.float32)
    for k in range(NC):
        idx = {0: np.zeros(NLO * 128, dtype=np.int16),
               1: np.zeros(NHI * 128, dtype=np.int16)}
        sval = {0: np.zeros((NLO, 128, WD), dtype=np.float32),
                1: np.zeros((NHI, 128, WD), dtype=np.float32)}
        cpos = {0: 0, 1: 0}
        for w in range(NW):
            for h in (0, 1):
                kk = (k * NW + w) * 2 + h
                s, e = starts[kk], ends[kk]
                n = e - s
                nchunks = int(nch[w, h])
                base = cpos[h]
                if n > 0:
                    esrc = allsrc[s:e] - (cfg.HALF if h else 0)
                    edl = dloc[s:e] - w * WD
                    pos = np.arange(n)
                    cidx = base + pos // 128
                    eidx = pos % 128
                    idx[h][(base * 128):(base * 128 + n)] = esrc.astype(np.int16)
                    sval[h][cidx, eidx, edl] = 1.0
                cpos[h] = base + nchunks
        # S stream layout: [128, nchunk*WD] bf16, [e, c*WD+d] = sval[c, e, d]
        slo = np.ascontiguousarray(sval[0].transpose(1, 0, 2).reshape(128, NLO * WD)).astype(BF16)
        shi = np.ascontiguousarray(sval[1].transpose(1, 0, 2).reshape(128, NHI * WD)).astype(BF16)

        dl = dinv[k * NPC:(k + 1) * NPC]
        dpad = np.zeros(cfg.PADD, dtype=np.float32)
        dpad[:NPC] = dl
        # [p, c] = dinv_local[c*128+p]
        dinv_cols = np.ascontiguousarray(dpad.reshape(cfg.NDCH, 128).T)
        dinv_rep = np.zeros((128, cfg.PADD), dtype=np.float32)
        dinv_rep[:, :NPC] = dl[None, :]
        dinv_rep = dinv_rep.astype(BF16)

        x_local = np.zeros((cfg.PADD, 128), dtype=np.float32)
        x_local[:NPC] = xf[k * NPC:(k + 1) * NPC]

        in_maps.append({
            "x_local": x_local,
            "idx_lo": _wrap_idx(idx[0]),
            "idx_hi": _wrap_idx(idx[1]),
            "s_lo": slo,
            "s_hi": shi,
            "dinv_cols": dinv_cols,
            "dinv_rep": dinv_rep,
            "w1": np.ascontiguousarray(W1b),
            "w2sb": W2sb,
            "bnc": bnc,
            "ident": ident,
        })
    return in_maps, sched


def build_program(cfg: Cfg, sched):
    N, NC = cfg.N, cfg.NC
    NW, PADD, NDCH, HALF = cfg.NW, cfg.PADD, cfg.NDCH, cfg.HALF
    NPC = cfg.NPC
    nlo_w, nhi_w = sched["nlo_w"], sched["nhi_w"]
    NLO, NHI = sched["NLO"], sched["NHI"]
    bf = mybir.dt.bfloat16
    f32 = mybir.dt.float32

    nc = bacc.Bacc("TRN2", target_bir_lowering=False, debug=False, num_devices=NC)

    x_local = nc.dram_tensor("x_local", [PADD, 128], f32, kind="ExternalInput")
    idx_lo = nc.dram_tensor("idx_lo", [128, max(NLO * 8, 16)], mybir.dt.int16, kind="ExternalInput")
    idx_hi = nc.dram_tensor("idx_hi", [128, max(NHI * 8, 16)], mybir.dt.int16, kind="ExternalInput")
    s_lo = nc.dram_tensor("s_lo", [128, max(NLO * WD, 64)], bf, kind="ExternalInput")
    s_hi = nc.dram_tensor("s_hi", [128, max(NHI * WD, 64)], bf, kind="ExternalInput")
    dinv_cols = nc.dram_tensor("dinv_cols", [128, NDCH], f32, kind="ExternalInput")
    dinv_rep_d = nc.dram_tensor("dinv_rep", [128, PADD], bf, kind="ExternalInput")
    w1_d = nc.dram_tensor("w1", [128, 256], bf, kind="ExternalInput")
    w2_d = nc.dram_tensor("w2sb", [128, 256], bf, kind="ExternalInput")
    bnc_d = nc.dram_tensor("bnc", [128, 9], f32, kind="ExternalInput")
    ident_d = nc.dram_tensor("ident", [128, 128], bf, kind="ExternalInput")
    x3_out = nc.dram_tensor("x3p", [128, 128], f32, kind="ExternalOutput")

    AF = mybir.ActivationFunctionType
    RG = [list(range(NC))]

    with tile.TileContext(nc) as tc:
        nc.gpsimd.load_library(library_config.mlp)
        with tc.tile_pool(name="consts", bufs=1) as consts, \
             tc.tile_pool(name="persist", bufs=1) as persist, \
             tc.tile_pool(name="dram", bufs=1, space="DRAM") as dram:

            idxlo_t = consts.tile([128, max(NLO * 8, 16)], mybir.dt.int16)
            idxhi_t = consts.tile([128, max(NHI * 8, 16)], mybir.dt.int16)
            nc.sync.dma_start(idxlo_t[:], idx_lo[:])
            nc.sync.dma_start(idxhi_t[:], idx_hi[:])
            dinvc_t = consts.tile([128, NDCH], f32)
            nc.sync.dma_start(dinvc_t[:], dinv_cols[:])
            dinvr_t = consts.tile([128, PADD], bf)
            nc.sync.dma_start(dinvr_t[:], dinv_rep_d[:])
            w1_t = consts.tile([128, 256], bf)
            nc.sync.dma_start(w1_t[:], w1_d[:])
            w2_t = consts.tile([128, 256], bf)
            nc.sync.dma_start(w2_t[:], w2_d[:])
            bnc_t = consts.tile([128, 9], f32)
            nc.sync.dma_start(bnc_t[:], bnc_d[:])
            ident_t = consts.tile([128, 128], bf)
            nc.sync.dma_start(ident_t[:], ident_d[:])

            # ---- x-tilde table: scale local x rows by dinv, cast bf16, AG ----
            xt_bounce = dram.tile([PADD, 128], bf)
            xt_table = dram.tile([N, 128], bf, addr_space="Shared")
            with tc.tile_pool(name="xb", bufs=3) as xb:
                for c in range(NDCH):
                    xt_in = xb.tile([128, 128], f32, tag="xt_in")
                    nc.sync.dma_start(xt_in[:], x_local[c * 128:(c + 1) * 128, :])
                    xt_o = xb.tile([128, 128], bf, tag="xt_o")
                    nc.scalar.activation(xt_o[:], xt_in[:], AF.Copy,
                                         scale=dinvc_t[:, c:c + 1])
                    nc.sync.dma_start(xt_bounce[c * 128:(c + 1) * 128, :], xt_o[:])
            nc.gpsimd.collective_compute(
                "AllGather", mybir.AluOpType.bypass, replica_groups=RG,
                ins=[xt_bounce[0:NPC, :].opt()], outs=[xt_table.opt()])
            xt_hi = dram.tile([HALF, 128], bf)
            nc.sync.dma_start(xt_hi[:], xt_table[HALF:2 * HALF, :])

            # ---- shared aggregation routine ----
            def aggregate(table_lo, table_hi, z_out, z_dtype):
                """z_out[:, :] (bf16/f32 [128, PADD]) = dinv_rep * (M.T @ S)"""
                with tc.tile_pool(name="glo", bufs=2) as glo_p, \
                     tc.tile_pool(name="ghi", bufs=2) as ghi_p, \
                     tc.tile_pool(name="slo", bufs=2) as slo_p, \
                     tc.tile_pool(name="shi", bufs=2) as shi_p, \
                     tc.tile_pool(name="zps", bufs=4, space="PSUM") as zps_p:
                    tiles = {0: {}, 1: {}}
                    gathered = {0: 0, 1: 0}
                    npad = {0: NLO, 1: NHI}
                    idxs = {0: idxlo_t, 1: idxhi_t}
                    s_d = {0: s_lo, 1: s_hi}
                    gp = {0: glo_p, 1: ghi_p}
                    sp = {0: slo_p, 1: shi_p}
                    tab = {0: table_lo[0:HALF, :], 1: table_hi[0:HALF, :]}

                    def ensure(h, c):
                        g = c // GROUP
                        if g in tiles[h]:
                            return tiles[h][g]
                        size = min(GROUP, npad[h] - g * GROUP)
                        mt = gp[h].tile([128, size, 128], bf, tag=f"m{h}",
                                        name=f"m{h}_{g}")
                        nc.gpsimd.dma_gather(
                            mt[:], tab[h], idxs[h][:, g * GROUP * 8:(g * GROUP + size) * 8],
                            size * 128, size * 128, 128)
                        st = sp[h].tile([128, size * WD], bf, tag=f"s{h}",
                                        name=f"s{h}_{g}")
                        nc.sync.dma_start(
                            st[:], s_d[h][:, g * GROUP * WD:(g * GROUP + size) * WD])
                        tiles[h][g] = (mt, st, g * GROUP)
                        gathered[h] = g * GROUP + size
                        return tiles[h][g]

                    pos = {0: 0, 1: 0}
                    for w in range(NW):
                        nch = {0: int(nlo_w[w]), 1: int(nhi_w[w])}
                        tot = nch[0] + nch[1]
                        if tot == 0:
                            continue
                        zt = zps_p.tile([128, WD], f32, tag="zt", name=f"z_{w}")
                        done = 0
                        for h in (0, 1):
                            for j in range(nch[h]):
                                c = pos[h] + j
                                mt, st, base = ensure(h, c)
                                slot = c - base
                                nc.tensor.matmul(
                                    zt[:], mt[:, slot, :],
                                    st[:, slot * WD:(slot + 1) * WD],
                                    start=(done == 0), stop=(done == tot - 1))
                                done += 1
                            pos[h] += nch[h]
                        nc.vector.tensor_tensor(
                            z_out[:, w * WD:(w + 1) * WD], zt[:],
                            dinvr_t[:, w * WD:(w + 1) * WD],
                            mybir.AluOpType.mult)

            # ---- layer 1 ----
            z1_t = persist.tile([128, PADD], bf)
            aggregate(xt_table, xt_hi, z1_t, bf)

            x1_t = persist.tile([128, 2, PADD], bf)     # [f1half, h, d]
            with tc.tile_pool(name="d1", bufs=3) as d1_p, \
                 tc.tile_pool(name="d1ps", bufs=3, space="PSUM") as d1ps:
                nblk = (PADD + 511) // 512
                for b in range(nblk):
                    d0 = b * 512
                    dsz = min(512, PADD - d0)
                    for hh in range(2):
                        hp = d1ps.tile([128, dsz], f32, tag="hps", name=f"h1_{b}_{hh}")
                        nc.tensor.matmul(hp[:], w1_t[:, hh * 128:(hh + 1) * 128],
                                         z1_t[:, d0:d0 + dsz], start=True, stop=True)
                        u = d1_p.tile([128, dsz], bf, tag="u", name=f"u_{b}_{hh}")
                        nc.scalar.activation(u[:], hp[:], AF.Relu,
                                             bias=bnc_t[:, 4 + hh:5 + hh])
                        nc.scalar.activation(x1_t[:, hh, d0:d0 + dsz], u[:], AF.Sigmoid,
                                             scale=bnc_t[:, 0 + hh:1 + hh],
                                             bias=bnc_t[:, 2 + hh:3 + hh])

            # ---- dense 2: h2 = x1 @ W2 (node-major), scale by dinv -> table ----
            ht_bounce = dram.tile([PADD, 128], bf)
            ht_table = dram.tile([N, 128], bf, addr_space="Shared")
            with tc.tile_pool(name="d2", bufs=3) as d2_p, \
                 tc.tile_pool(name="d2ps", bufs=3, space="PSUM") as d2ps:
                for c in range(NDCH):
                    hp = d2ps.tile([128, 128], f32, tag="h2ps", name=f"h2_{c}")
                    for hh in range(2):
                        nc.tensor.matmul(hp[:], x1_t[:, hh, c * 128:(c + 1) * 128],
                                         w2_t[:, hh * 128:(hh + 1) * 128],
                                         start=(hh == 0), stop=(hh == 1))
                    ho = d2_p.tile([128, 128], bf, tag="ho", name=f"ho_{c}")
                    nc.scalar.activation(ho[:], hp[:], AF.Copy,
                                         scale=dinvc_t[:, c:c + 1])
                    nc.sync.dma_start(ht_bounce[c * 128:(c + 1) * 128, :], ho[:])
            nc.gpsimd.collective_compute(
                "AllGather", mybir.AluOpType.bypass, replica_groups=RG,
                ins=[ht_bounce[0:NPC, :].opt()], outs=[ht_table.opt()])
            ht_hi = dram.tile([HALF, 128], bf)
            nc.sync.dma_start(ht_hi[:], ht_table[HALF:2 * HALF, :])

            # ---- layer 2 ----
            z2_t = persist.tile([128, PADD], bf)
            aggregate(ht_table, ht_hi, z2_t, bf)

            x2_t = persist.tile([128, PADD], bf)
            with tc.tile_pool(name="l2a", bufs=3) as l2a:
                nblk = (PADD + 511) // 512
                for b in range(nblk):
                    d0 = b * 512
                    dsz = min(512, PADD - d0)
                    v = l2a.tile([128, dsz], bf, tag="v", name=f"v_{b}")
                    nc.scalar.activation(v[:], z2_t[:, d0:d0 + dsz], AF.Relu,
                                         bias=bnc_t[:, 6:7])
                    nc.scalar.activation(x2_t[:, d0:d0 + dsz], v[:], AF.Sigmoid,
                                         scale=bnc_t[:, 7:8], bias=bnc_t[:, 8:9])
            if PADD > NPC:
                nc.vector.memset(x2_t[:, NPC:PADD], 0.0)

            # ---- final: x3 = sum_d x2[:, d] (x) x2[:, d] ----
            with tc.tile_pool(name="fin", bufs=3) as fin, \
                 tc.tile_pool(name="finps", bufs=3, space="PSUM") as finps, \
                 tc.tile_pool(name="x3ps", bufs=1, space="PSUM") as x3ps:
                x3p = x3ps.tile([128, 128], f32)
                for c in range(NDCH):
                    tp = finps.tile([128, 128], bf, tag="tp", name=f"tp_{c}")
                    nc.tensor.transpose(tp[:], x2_t[:, c * 128:(c + 1) * 128], ident_t[:])
                    x2n = fin.tile([128, 128], bf, tag="x2n", name=f"x2n_{c}")
                    nc.scalar.copy(x2n[:], tp[:])
                    nc.tensor.matmul(x3p[:], x2n[:], x2n[:],
                                     start=(c == 0), stop=(c == NDCH - 1))
                x3s = fin.tile([128, 128], f32, tag="x3s")
                nc.scalar.copy(x3s[:], x3p[:])
                nc.sync.dma_start(x3_out[:], x3s[:])

    nc.compile()
    return nc


def ref_numpy(x, edge_index, W1, b1, W2, b2, g1, be1, m1, v1, g2, be2, m2, v2):
    """fp32 numpy mirror of reference.py."""
    x = np.asarray(x, np.float32)
    src = np.asarray(edge_index[0], np.int64)
    dst = np.asarray(edge_index[1], np.int64)
    N = x.shape[0]
    deg = np.bincount(dst, minlength=N).astype(np.float32) + 1.0
    dinv = 1.0 / np.sqrt(deg)

    def conv(xi, W, b):
        h = xi @ W
        coef = (dinv[src] * dinv[dst])[:, None]
        agg = np.zeros_like(h)
        np.add.at(agg, dst, h[src] * coef)
        agg += (dinv * dinv)[:, None] * h
        return agg + b

    def bn(xi, g, be, m, v):
        return (xi - m) / np.sqrt(v + BN_EPS) * g + be

    def sig(a):
        return 1.0 / (1.0 + np.exp(-a))

    h = np.maximum(conv(x, W1, b1), 0.0)
    x1 = sig(bn(h, g1, be1, m1, v1))
    h2 = np.maximum(conv(x1, W2, b2), 0.0)
    x2 = sig(bn(h2, g2, be2, m2, v2))
    return x2.T @ x2


# ---------------------------------------------------------------------------
# harness entry point
# ---------------------------------------------------------------------------
_CACHE = {}


def kernel(x, edge_index, W1, b1, W2, b2, g1, be1, m1, v1, g2, be2, m2, v2,
           W3=None, b3=None, **_unused):
    """Full (unsharded) inputs in, full [128,128] float32 output out."""
    cfg = Cfg(50000, 8)
    in_maps, sched = prep_host(x, edge_index, W1, b1, W2, b2,
                               g1, be1, m1, v1, g2, be2, m2, v2, cfg)
    key = (sched["NLO"], sched["NHI"], tuple(sched["nlo_w"]), tuple(sched["nhi_w"]))
    if key not in _CACHE:
        _CACHE[key] = build_program(cfg, sched)
    nc = _CACHE[key]
    res = run_bass_kernel_spmd(nc, in_maps, core_ids=list(range(8)))
    x3 = sum(np.asarray(res.results[k]["x3p"], np.float64) for k in range(8))
    return x3.astype(np.float32)



# revision 2
# speedup vs baseline: 1.0207x; 1.0207x over previous
"""GCN message-passing kernel for TRN2, 8-core SPMD — v4.

Changes over v3:
  - x-tilde prescaled on host (bf16 input), AllGather starts at t~0.
  - One-hot S matrices host-built, streamed from DRAM per call (no DVE
    is_equal; DVE only does the per-window PSUM evict multiply).
  - Gather calls span 4-window groups, split lo/hi: ~26 calls/layer.
  - dense1 + dense2 + h-tilde bounce fused per 4-window group inside the
    L1 aggregation loop; BN2+sigmoid+x3-accumulate fused into L2 loop.
"""
import math
import numpy as np
import ml_dtypes

import concourse.bacc as bacc
import concourse.bass as bass
import concourse.mybir as mybir
import concourse.tile as tile
from concourse import library_config
from concourse.bass_utils import run_bass_kernel_spmd

BF16 = ml_dtypes.bfloat16
F_IN, F_HID, F_OUT = 128, 256, 128
BN_EPS = 1e-3
WD = 128
GW = 4               # windows per group
NQ = 4               # SWDGE queues round-robin
N, NC = 50000, 8
NPC = N // NC
NDCH = math.ceil(NPC / 128)
PADD = NDCH * 128
NW = PADD // WD      # 49
NG = math.ceil(NW / GW)
LO_END = 32768
HI_BEG = N - 32768


def _wrap_idx(idx_flat):
    w = idx_flat.reshape(-1, 16).T.astype(np.int16)
    return np.ascontiguousarray(np.tile(w, (8, 1)))


def prep_host(x, edge_index, W1, b1, W2, b2, g1, be1, m1, v1, g2, be2, m2, v2):
    src = np.asarray(edge_index[0], dtype=np.int64)
    dst = np.asarray(edge_index[1], dtype=np.int64)

    deg = np.bincount(dst, minlength=N).astype(np.float64) + 1.0
    dinv = (1.0 / np.sqrt(deg)).astype(np.float32)

    allsrc = np.concatenate([src, np.arange(N, dtype=np.int64)])
    alldst = np.concatenate([dst, np.arange(N, dtype=np.int64)])
    core = alldst // NPC
    dloc = alldst % NPC
    win = dloc // WD

    # balanced lo/hi half assignment (overlap band [HI_BEG, LO_END))
    half = np.where(allsrc < HI_BEG, 0,
                    np.where(allsrc >= LO_END, 1, -1)).astype(np.int64)
    gid = core * NW + win
    fixed0 = np.bincount(gid[half == 0], minlength=NC * NW)
    fixed1 = np.bincount(gid[half == 1], minlength=NC * NW)
    flex = np.flatnonzero(half == -1)
    fgid = gid[flex]
    o = np.argsort(fgid, kind="stable")
    flex = flex[o]
    fgid = fgid[o]
    nf = np.bincount(fgid, minlength=NC * NW)
    to0 = np.clip((nf + fixed1 - fixed0 + 1) // 2, 0, nf)
    gstart = np.concatenate([[0], np.cumsum(nf)[:-1]])
    rank_in_g = np.arange(flex.size) - gstart[fgid]
    half[flex] = np.where(rank_in_g < to0[fgid], 0, 1)

    order = np.lexsort((dloc, half, win, core))
    csrc, ccore, cdloc = allsrc[order], core[order], dloc[order]
    cwin, chalf = win[order], half[order]

    cnt = np.zeros((NC, NW, 2), dtype=np.int64)
    np.add.at(cnt, (ccore, cwin, chalf), 1)
    need_h = np.ceil(cnt / 128).astype(np.int64).max(axis=0)   # [NW, 2]
    NCH = int(need_h.sum())

    key = (ccore * NW + cwin) * 2 + chalf
    all_keys = np.arange(NC * NW * 2)
    starts = np.searchsorted(key, all_keys, side="left")
    ends = np.searchsorted(key, all_keys, side="right")

    # chunk slot layout: per group g: [lo chunks of windows g*4..g*4+3,
    # then hi chunks of those windows]
    cbase = {}
    b = 0
    for g in range(NG):
        ws = range(g * GW, min((g + 1) * GW, NW))
        for h in (0, 1):
            for w in ws:
                cbase[(w, h)] = b
                b += int(need_h[w, h])
    assert b == NCH

    # folded BN constants
    A1 = (g1 * (1.0 / np.sqrt(v1 + BN_EPS))).astype(np.float32)
    B1 = (be1 - m1 * A1).astype(np.float32)
    A2 = (g2 * (1.0 / np.sqrt(v2 + BN_EPS))).astype(np.float32)
    B2 = (be2 - m2 * A2).astype(np.float32)
    bnc = np.zeros((128, 9), dtype=np.float32)
    bnc[:, 0], bnc[:, 1] = A1[:128], A1[128:]
    bnc[:, 2], bnc[:, 3] = B1[:128], B1[128:]
    bnc[:, 4], bnc[:, 5] = b1[:128], b1[128:]
    bnc[:, 6], bnc[:, 7], bnc[:, 8] = b2, A2, B2

    W1b = np.asarray(W1, dtype=np.float32).astype(BF16)
    W2f = np.asarray(W2, dtype=np.float32)
    W2sb = np.zeros((128, 256), dtype=np.float32)
    W2sb[:, 0:128] = W2f[0:128, :]
    W2sb[:, 128:256] = W2f[128:256, :]
    W2sb = W2sb.astype(BF16)
    ident = np.eye(128, dtype=np.float32).astype(BF16)

    xf = np.asarray(x, dtype=np.float32)
    xt_all = (xf * dinv[:, None]).astype(BF16)        # prescaled x-tilde

    in_maps = []
    for k in range(NC):
        idxs = np.zeros(NCH * 128, dtype=np.int16)
        sval = np.zeros((128, NCH * WD), dtype=np.float32)
        for w in range(NW):
            for h in (0, 1):
                kk = (k * NW + w) * 2 + h
                lo, hi = starts[kk], ends[kk]
                n = hi - lo
                if n == 0:
                    continue
                vals = (csrc[lo:hi] - (HI_BEG if h else 0)).astype(np.int16)
                ed = (cdloc[lo:hi] - w * WD).astype(np.int64)
                c0 = cbase[(w, h)]
                pos = np.arange(n)
                ci = c0 + pos // 128
                si = pos % 128
                idxs[ci * 128 + si] = vals
                sval[si, ci * WD + ed] = 1.0

        dl = dinv[k * NPC:(k + 1) * NPC]
        dpad = np.zeros(PADD, dtype=np.float32)
        dpad[:NPC] = dl
        dinv_cols = np.ascontiguousarray(dpad.reshape(NDCH, 128).T)
        dinv_rep = np.zeros((128, PADD), dtype=np.float32)
        dinv_rep[:, :NPC] = dl[None, :]
        dinv_rep = dinv_rep.astype(BF16)

        in_maps.append({
            "xtl": np.ascontiguousarray(xt_all[k * NPC:(k + 1) * NPC]),
            "idxs": _wrap_idx(idxs),
            "s_d": np.ascontiguousarray(sval.astype(BF16)),
            "dinv_cols": dinv_cols,
            "dinv_rep": dinv_rep,
            "w1": np.ascontiguousarray(W1b),
            "w2sb": W2sb,
            "bnc": bnc,
            "ident": ident,
        })
    sched = {"NCH": NCH, "need_h": tuple(tuple(int(v) for v in r) for r in need_h)}
    return in_maps, sched


def build_program(sched):
    NCH = sched["NCH"]
    need_h = sched["need_h"]
    bf = mybir.dt.bfloat16
    f32 = mybir.dt.float32
    i16 = mybir.dt.int16
    TABN = 50048
    MAXC = max(sum(need_h[w][h] for w in range(g * GW, min((g + 1) * GW, NW)))
               for g in range(NG) for h in (0, 1))

    nc = bacc.Bacc("TRN2", target_bir_lowering=False, debug=False,
                   num_devices=NC, num_swdge_queues=NQ)

    xtl_d = nc.dram_tensor("xtl", [NPC, 128], bf, kind="ExternalInput")
    idxs_d = nc.dram_tensor("idxs", [128, NCH * 8], i16, kind="ExternalInput")
    s_dd = nc.dram_tensor("s_d", [128, NCH * WD], bf, kind="ExternalInput")
    dinv_cols = nc.dram_tensor("dinv_cols", [128, NDCH], f32, kind="ExternalInput")
    dinv_rep_d = nc.dram_tensor("dinv_rep", [128, PADD], bf, kind="ExternalInput")
    w1_d = nc.dram_tensor("w1", [128, 256], bf, kind="ExternalInput")
    w2_d = nc.dram_tensor("w2sb", [128, 256], bf, kind="ExternalInput")
    bnc_d = nc.dram_tensor("bnc", [128, 9], f32, kind="ExternalInput")
    ident_d = nc.dram_tensor("ident", [128, 128], bf, kind="ExternalInput")
    x3_out = nc.dram_tensor("x3p", [128, 128], f32, kind="ExternalOutput")

    AF = mybir.ActivationFunctionType
    ALU = mybir.AluOpType
    RG = [list(range(NC))]

    with tile.TileContext(nc) as tc:
        nc.gpsimd.load_library(library_config.mlp)
        with tc.tile_pool(name="consts", bufs=1) as consts, \
             tc.tile_pool(name="persist", bufs=1) as persist, \
             tc.tile_pool(name="dram", bufs=1, space="DRAM") as dram:

            xt_table = dram.tile([TABN, 128], bf, addr_space="Shared")
            xt_bounce = dram.tile([NPC, 128], bf)
            nc.sync.dma_start(xt_bounce[:, :], xtl_d[:, :])
            nc.gpsimd.collective_compute(
                "AllGather", mybir.AluOpType.bypass, replica_groups=RG,
                ins=[xt_bounce[:, :].opt()], outs=[xt_table[0:N, :].opt()])

            idxs_t = consts.tile([128, NCH * 8], i16)
            nc.sync.dma_start(idxs_t[:], idxs_d[:])
            dinvc_t = consts.tile([128, NDCH], f32)
            nc.sync.dma_start(dinvc_t[:], dinv_cols[:])
            dinvr_t = consts.tile([128, PADD], bf)
            nc.sync.dma_start(dinvr_t[:], dinv_rep_d[:])
            w1_t = consts.tile([128, 256], bf)
            nc.sync.dma_start(w1_t[:], w1_d[:])
            w2_t = consts.tile([128, 256], bf)
            nc.sync.dma_start(w2_t[:], w2_d[:])
            bnc_t = consts.tile([128, 9], f32)
            nc.sync.dma_start(bnc_t[:], bnc_d[:])
            ident_t = consts.tile([128, 128], bf)
            nc.sync.dma_start(ident_t[:], ident_d[:])

            ht_bounce = dram.tile([PADD, 128], bf)
            ht_table = dram.tile([TABN, 128], bf, addr_space="Shared")

            def aggregate(table_dram, z_out, per_group_hook):
                """Gather+S-matmul per 4-window group; hook(g, w_list) runs
                after the group's windows are evicted."""
                qn = [0]
                c0 = [0]
                with tc.tile_pool(name="gbuf", bufs=10) as gbuf, \
                     tc.tile_pool(name="sp", bufs=6) as sp, \
                     tc.tile_pool(name="zps", bufs=GW, space="PSUM") as zps:
                    for g in range(NG):
                        ws = list(range(g * GW, min((g + 1) * GW, NW)))
                        zts = {w: zps.tile([128, WD], f32, tag="zt",
                                           name=f"z_{w}") for w in ws}
                        done = {w: 0 for w in ws}
                        tot = {w: need_h[w][0] + need_h[w][1] for w in ws}
                        for h in (0, 1):
                            nch = sum(need_h[w][h] for w in ws)
                            if nch == 0:
                                continue
                            tab = (table_dram[0:LO_END, :] if h == 0 else
                                   table_dram[HI_BEG:HI_BEG + LO_END, :])
                            # sub-calls of <= 8 chunks (1024 idxs: SWDGE ring cap)
                            tiles = []
                            for sc0 in range(0, nch, 8):
                                scn = min(8, nch - sc0)
                                cc = c0[0] + sc0
                                g_t = gbuf.tile([128, 8, 128], bf, tag="g",
                                                name=f"g_{g}_{h}_{sc0}")
                                nc.gpsimd.dma_gather(
                                    g_t[:, 0:scn, :], tab,
                                    idxs_t[:, cc * 8:(cc + scn) * 8],
                                    scn * 128, scn * 128, 128,
                                    queue_num=qn[0] % NQ)
                                qn[0] += 1
                                s_t = sp.tile([128, 8 * WD], bf, tag="s",
                                              name=f"s_{g}_{h}_{sc0}")
                                nc.sync.dma_start(
                                    s_t[:, 0:scn * WD],
                                    s_dd[:, cc * WD:(cc + scn) * WD])
                                tiles.append((g_t, s_t))
                            j = 0
                            for w in ws:
                                for _ in range(need_h[w][h]):
                                    g_t, s_t = tiles[j // 8]
                                    sl = j % 8
                                    nc.tensor.matmul(
                                        zts[w][:], g_t[:, sl, :],
                                        s_t[:, sl * WD:(sl + 1) * WD],
                                        start=(done[w] == 0),
                                        stop=(done[w] == tot[w] - 1))
                                    done[w] += 1
                                    j += 1
                            c0[0] += nch
                        for w in ws:
                            nc.vector.tensor_tensor(
                                z_out[:, w * WD:(w + 1) * WD], zts[w][:],
                                dinvr_t[:, w * WD:(w + 1) * WD], ALU.mult)
                        per_group_hook(g, ws)
                assert c0[0] == NCH

            # ---- layer 1 (dense1+dense2+bounce fused per group) ----
            z1_t = persist.tile([128, PADD], bf)
            x1_t = persist.tile([128, 2, PADD], bf)

            d1_p = tc.alloc_tile_pool(name="d1", bufs=3)
            d1ps = tc.alloc_tile_pool(name="d1ps", bufs=2, space="PSUM")
            d2_p = tc.alloc_tile_pool(name="d2", bufs=3)
            d2ps = tc.alloc_tile_pool(name="d2ps", bufs=2, space="PSUM")

            def l1_hook(g, ws):
                d0 = ws[0] * WD
                dsz = len(ws) * WD
                for hh in range(2):
                    hp = d1ps.tile([128, 512], f32, tag="hps",
                                   name=f"h1_{g}_{hh}")
                    nc.tensor.matmul(hp[:, 0:dsz], w1_t[:, hh * 128:(hh + 1) * 128],
                                     z1_t[:, d0:d0 + dsz], start=True, stop=True)
                    u = d1_p.tile([128, 512], bf, tag="u", name=f"u_{g}_{hh}")
                    nc.scalar.activation(u[:, 0:dsz], hp[:, 0:dsz], AF.Relu,
                                         bias=bnc_t[:, 4 + hh:5 + hh])
                    nc.scalar.activation(x1_t[:, hh, d0:d0 + dsz], u[:, 0:dsz],
                                         AF.Sigmoid,
                                         scale=bnc_t[:, 0 + hh:1 + hh],
                                         bias=bnc_t[:, 2 + hh:3 + hh])
                for w in ws:   # dense2 per 128-node chunk (chunk == window)
                    hp2 = d2ps.tile([128, 128], f32, tag="h2ps", name=f"h2_{w}")
                    for hh in range(2):
                        nc.tensor.matmul(hp2[:], x1_t[:, hh, w * 128:(w + 1) * 128],
                                         w2_t[:, hh * 128:(hh + 1) * 128],
                                         start=(hh == 0), stop=(hh == 1))
                    ho = d2_p.tile([128, 128], bf, tag="ho", name=f"ho_{w}")
                    nc.scalar.activation(ho[:], hp2[:], AF.Copy,
                                         scale=dinvc_t[:, w:w + 1])
                    nc.sync.dma_start(ht_bounce[w * 128:(w + 1) * 128, :], ho[:])

            aggregate(xt_table, z1_t[:], l1_hook)
            d2ps.release()
            d2_p.release()
            d1ps.release()
            d1_p.release()

            nc.gpsimd.collective_compute(
                "AllGather", mybir.AluOpType.bypass, replica_groups=RG,
                ins=[ht_bounce[0:NPC, :].opt()], outs=[ht_table[0:N, :].opt()])

            # ---- layer 2 (BN2+sigmoid+x3 accumulate fused per group) ----
            z2_t = z1_t
            x2_t = x1_t[:, 0, :]

            l2a = tc.alloc_tile_pool(name="l2a", bufs=3)
            fin = tc.alloc_tile_pool(name="fin", bufs=3)
            finps = tc.alloc_tile_pool(name="finps", bufs=2, space="PSUM")
            x3ps = tc.alloc_tile_pool(name="x3ps", bufs=1, space="PSUM")
            x3p = x3ps.tile([128, 128], f32)

            def l2_hook(g, ws):
                d0 = ws[0] * WD
                dsz = len(ws) * WD
                v = l2a.tile([128, 512], bf, tag="v", name=f"v_{g}")
                nc.scalar.activation(v[:, 0:dsz], z2_t[:, d0:d0 + dsz], AF.Relu,
                                     bias=bnc_t[:, 6:7])
                nc.scalar.activation(x2_t[:, d0:d0 + dsz], v[:, 0:dsz], AF.Sigmoid,
                                     scale=bnc_t[:, 7:8], bias=bnc_t[:, 8:9])
                if ws[-1] == NW - 1 and PADD > NPC:
                    nc.vector.memset(x2_t[:, NPC:PADD], 0.0)
                for w in ws:
                    tp = finps.tile([128, 128], bf, tag="tp", name=f"ftp_{w}")
                    nc.tensor.transpose(tp[:], x2_t[:, w * 128:(w + 1) * 128],
                                        ident_t[:])
                    x2n = fin.tile([128, 128], bf, tag="x2n", name=f"x2n_{w}")
                    nc.scalar.copy(x2n[:], tp[:])
                    nc.tensor.matmul(x3p[:], x2n[:], x2n[:],
                                     start=(w == 0), stop=(w == NW - 1))

            aggregate(ht_table, z2_t[:], l2_hook)

            x3s = fin.tile([128, 128], f32, tag="x3s")
            nc.scalar.copy(x3s[:], x3p[:])
            nc.sync.dma_start(x3_out[:], x3s[:])
            x3ps.release()
            finps.release()
            fin.release()
            l2a.release()

    nc.compile()
    return nc


_CACHE = {}


def kernel(x, edge_index, W1, b1, W2, b2, g1, be1, m1, v1, g2, be2, m2, v2,
           W3=None, b3=None, **_unused):
    in_maps, sched = prep_host(x, edge_index, W1, b1, W2, b2,
                               g1, be1, m1, v1, g2, be2, m2, v2)
    key = (sched["NCH"], sched["need_h"])
    if key not in _CACHE:
        _CACHE[key] = build_program(sched)
    nc = _CACHE[key]
    res = run_bass_kernel_spmd(nc, in_maps, core_ids=list(range(8)))
    x3 = sum(np.asarray(res.results[k]["x3p"], np.float64) for k in range(8))
    return x3.astype(np.float32)
